# revision 1
# baseline (speedup 1.0000x reference)
"""Bahdanau-attention kernel for one TRN2 chip (8 NeuronCores, SPMD).

Math (per batch row b, sequence position s):
    att[b, s] = v . tanh(h_part[b] + enc[s, b, :] @ W_e)
    out[b, :] = softmax(att[b, :])        with h_part = hidden @ W_h + b_attn

Sharding: pure data-parallel over batch (B=32 -> 4 per core), no collectives.

Key design points:
- Host-side layout prep: the big matmul contracts over H, which must live on
  SBUF partitions, so encoder_outputs is pre-transposed to H-major on the host
  and every device DMA is one contiguous block.
- The energy matmul runs as fp8(e4m3) DoubleRow (2 weights/cell, effective
  K=256 per pass, half the matmul count of bf16).  W_e is pre-scaled by 64 on
  the host so its small values stay in fp8's normal range; the tanh activation
  rescales by 1/64 for free.  h_part / v-dot stay bf16; accumulation is fp32.
- tanh(h_part + e_part) runs on the scalar engine with the per-(q,b) bias
  folded in; [128,1024] tiles halve the per-op overhead.  Softmax skips the
  max-subtraction (|logit| <= ||v||_1 ~ 18, safe in fp32 exp).
- Software-pipelined emission: e-matmuls of block i+1 precede the
  tanh-dependent v-dot matmuls of block i-1 in the PE stream (2-block skew),
  exp is deferred one block so it never head-of-line-blocks tanh in the ACT
  FIFO, and dummy matmuls pre-warm the PE clock (HAM) during the first DMAs.
Measured: ~78 us on-chip (neuron-profile exec_time), rel err ~1.3e-2 vs the
fp32 reference (L2); max abs err ~6e-5 on a softmax output of scale ~0.1.
"""

import sys

sys.path.insert(0, "/opt/trn_rl_repo")

import numpy as np

from concourse import bacc, bass, mybir, tile
from concourse.bass_utils import run_bass_kernel_spmd

H = 512
DH = 4 * H            # 2048 (hidden feature dim)
B, S = 32, 2048
NCORES = 8
BC = B // NCORES      # 4 batch rows per core
KH = H // 128         # 4 contraction tiles over H
KD = DH // 128        # 16 contraction tiles over DH
NQ = H // 128         # 4 output quadrants of H
SBLK = 1024           # sequence positions per block
NBLK = S // SBLK      # 2 blocks per batch row
HB = 512              # half-block: psum-bank / matmul-N granularity
NCH = S // HB         # 4 per-row chunks for the softmax
F32 = mybir.dt.float32
F32R = mybir.dt.float32r
BF16 = mybir.dt.bfloat16
F8 = mybir.dt.float8e4
WE_SCALE = 64.0

_NC_CACHE = None


def _build():
    nc = bacc.Bacc(
        "TRN2", target_bir_lowering=False, debug=False, num_devices=NCORES
    )
    enc_d = nc.dram_tensor(
        "enc_t", [BC, NBLK, 128, KH, SBLK], F8, kind="ExternalInput"
    )
    hid_d = nc.dram_tensor("hid_t", [128, KD, BC], BF16, kind="ExternalInput")
    wh_d = nc.dram_tensor("w_h", [128, KD, H], BF16, kind="ExternalInput")
    we_d = nc.dram_tensor("w_e", [128, KH, H], F8, kind="ExternalInput")
    ba_d = nc.dram_tensor("b_attn", [128, NQ], F32, kind="ExternalInput")
    v_d = nc.dram_tensor("v", [128, NQ], BF16, kind="ExternalInput")
    id_d = nc.dram_tensor("ident", [BC, BC], F32, kind="ExternalInput")
    out_d = nc.dram_tensor("out", [BC, S], F32, kind="ExternalOutput")

    TANH = mybir.ActivationFunctionType.Tanh
    EXP = mybir.ActivationFunctionType.Exp
    COPY = mybir.ActivationFunctionType.Copy

    with tile.TileContext(nc) as tc:
        with (
            tc.tile_pool(name="const", bufs=1) as constp,
            tc.tile_pool(name="enc", bufs=6) as encp,
            tc.tile_pool(name="energy", bufs=8) as enp,
            tc.tile_pool(name="small", bufs=1) as smallp,
            tc.tile_pool(name="psum_e", bufs=3, space=bass.MemorySpace.PSUM) as pse,
            tc.tile_pool(name="psum_s", bufs=1, space=bass.MemorySpace.PSUM) as pss,
        ):
            wh_sb = constp.tile([128, KD, H], BF16)
            nc.scalar.dma_start(wh_sb[:, 0 : KD // 2, :], wh_d[:, 0 : KD // 2, :])
            we_sb = constp.tile([128, KH, H], F8)
            for k in range(KH):
                nc.scalar.dma_start(we_sb[:, k, :], we_d[:, k, :])
            ba_sb = constp.tile([128, NQ], F32)
            nc.scalar.dma_start(ba_sb[:], ba_d[:])
            v_sb = constp.tile([128, NQ], BF16)
            nc.scalar.dma_start(v_sb[:], v_d[:])
            id_sb = constp.tile([BC, BC], F32)
            nc.scalar.dma_start(id_sb[:], id_d[:])

            hptb = constp.tile([128, NQ, BC], F32)
            ex = smallp.tile([128, S], F32)
            out_sb = smallp.tile([128, S], F32)
            esum = smallp.tile([128, NCH], F32)
            ssum = smallp.tile([128, 1], F32)
            rs = smallp.tile([128, 1], F32)

            ps_small = pss.tile([128, HB], F32)

            # HAM pre-warm: ~3.5 us of dummy matmuls on zeroed scratch while
            # the first DMAs are still in flight, so real matmuls start at
            # full clock (K=8/8)
            warm = constp.tile([128, 512], BF16)
            nc.vector.memset(warm[:], 0.0)
            for _ in range(8):
                nc.tensor.matmul(
                    ps_small[:, :], warm[:, 0:128], warm[:], start=True, stop=True
                )

            blocks = [(b, s) for b in range(BC) for s in range(NBLK)]
            NBLOCKS = len(blocks)
            ets = {}
            epss = {}

            def load_block(i):
                b, sblk = blocks[i]
                et = encp.tile([128, KH, SBLK], F8)
                nc.sync.dma_start(et[:], enc_d[b, sblk])
                ets[i] = et

            def emit_emm(i, qs=None):
                b, sblk = blocks[i]
                if qs is None or qs[0] == 0:
                    epss[i] = []
                et = ets[i]
                eps4 = epss[i]
                qlist = list(qs) if qs is not None else list(range(NQ))
                tiles = {}
                for q in qlist:
                    tiles[q] = pse.tile([128, SBLK], F32, name="eps", tag="eps")
                for qpair in [qlist[i : i + 2] for i in range(0, len(qlist), 2)]:
                    for half in range(SBLK // HB):
                        hsl = slice(half * HB, (half + 1) * HB)
                        for j in range(KH // 2):
                            for q in qpair:
                                nc.tensor.matmul(
                                    tiles[q][:, hsl],
                                    we_sb[
                                        :, 2 * j : 2 * j + 2, q * 128 : (q + 1) * 128
                                    ],
                                    et[:, 2 * j : 2 * j + 2, hsl],
                                    start=(j == 0),
                                    stop=(j == KH // 2 - 1),
                                    perf_mode=mybir.MatmulPerfMode.DoubleRow,
                                )
                for q in qlist:
                    eps4.append(tiles[q])
                if qs is None or qs[-1] == NQ - 1:
                    ets.pop(i)

            ens = {}

            def emit_tanh(i):
                b, sblk = blocks[i]
                en4 = []
                for q in range(NQ):
                    eps = epss[i][q]
                    en = enp.tile([128, SBLK], BF16)
                    nc.scalar.activation(
                        en[:],
                        eps[:],
                        TANH,
                        bias=hptb[:, q, b : b + 1],
                        scale=1.0 / WE_SCALE,
                    )
                    en4.append(en)
                ens[i] = en4
                del epss[i]

            def emit_v(i):
                for half in range(SBLK // HB):
                    c = i * (SBLK // HB) + half
                    att_ps = ps_small[(c % 3) * 32 : (c % 3) * 32 + 1, 0:HB]
                    for q in range(NQ):
                        nc.tensor.matmul(
                            att_ps,
                            v_sb[:, q : q + 1],
                            ens[i][q][:, half * HB : (half + 1) * HB],
                            start=(q == 0),
                            stop=(q == NQ - 1),
                        )
                del ens[i]

            def emit_exp(i):
                # exp of block i's logits (no max-sub: |logit| <= ||v||_1 ~ 18).
                # Deferred so it never head-of-line-blocks tanh in the ACT FIFO.
                b, sblk = blocks[i]
                r0 = b * 32
                for half in range(SBLK // HB):
                    emit_exp_chunk(i, b, sblk * (SBLK // HB) + half,
                                   i * (SBLK // HB) + half)
                if sblk == NBLK - 1:
                    emit_norm(i, b, r0)

            def emit_exp_chunk(i, b, c, cg):
                r0 = b * 32
                att_ps = ps_small[(cg % 3) * 32 : (cg % 3) * 32 + 1, 0:HB]
                if i >= NBLOCKS - 2:
                    # tail-critical: fused accumulator (290 ns) beats a
                    # separate 680 ns single-partition DVE reduce
                    nc.scalar.activation(
                        ex[r0 : r0 + 1, c * HB : (c + 1) * HB],
                        att_ps,
                        EXP,
                        accum_out=esum[r0 : r0 + 1, c : c + 1],
                    )
                else:
                    nc.scalar.activation(
                        ex[r0 : r0 + 1, c * HB : (c + 1) * HB],
                        att_ps,
                        EXP,
                    )
                    nc.vector.reduce_sum(
                        esum[r0 : r0 + 1, c : c + 1],
                        ex[r0 : r0 + 1, c * HB : (c + 1) * HB],
                        axis=mybir.AxisListType.X,
                    )

            def emit_norm(i, b, r0):
                if True:
                    # normalize row b as soon as its blocks are done
                    nc.vector.reduce_sum(
                        ssum[r0 : r0 + 1, :],
                        esum[r0 : r0 + 1, :],
                        axis=mybir.AxisListType.X,
                    )
                    nc.vector.reciprocal(rs[r0 : r0 + 1, :], ssum[r0 : r0 + 1, :])
                    if i == NBLOCKS - 1:
                        # last row: split across engines so the exposed tail
                        # is half as long
                        hs = S // 2
                        nc.vector.tensor_scalar_mul(
                            out_sb[r0 : r0 + 1, 0:hs],
                            ex[r0 : r0 + 1, 0:hs],
                            rs[r0 : r0 + 1, :],
                        )
                        nc.scalar.activation(
                            out_sb[r0 : r0 + 1, hs:S],
                            ex[r0 : r0 + 1, hs:S],
                            COPY,
                            scale=rs[r0 : r0 + 1, :],
                        )
                        nc.sync.dma_start(
                            out_d[b : b + 1, 0:hs], out_sb[r0 : r0 + 1, 0:hs]
                        )
                        nc.scalar.dma_start(
                            out_d[b : b + 1, hs:S], out_sb[r0 : r0 + 1, hs:S]
                        )
                    else:
                        nc.vector.tensor_scalar_mul(
                            out_sb[r0 : r0 + 1, :],
                            ex[r0 : r0 + 1, :],
                            rs[r0 : r0 + 1, :],
                        )
                        nc.sync.dma_start(
                            out_d[b : b + 1, :], out_sb[r0 : r0 + 1, :]
                        )

            # prologue: sync queue carries only enc tiles (fp8, 256 KB each);
            # h_part matmuls interleave with block 0's e-matmuls so the tanh
            # bias is ready as early as possible
            load_block(0)
            hid_sb = constp.tile([128, KD, BC], BF16)
            nc.sync.dma_start(hid_sb[:], hid_d[:])
            nc.sync.dma_start(wh_sb[:, KD // 2 :, :], wh_d[:, KD // 2 :, :])
            load_block(1)
            hp_ps = ps_small[0:BC, 0:H]

            def emit_hp(ks):
                for k in ks:
                    nc.tensor.matmul(
                        hp_ps,
                        hid_sb[:, k, :],
                        wh_sb[:, k, :],
                        start=(k == 0),
                        stop=(k == KD - 1),
                    )

            emit_hp(range(KD))
            hp_sb = smallp.tile([BC, H], F32)
            nc.vector.tensor_copy(hp_sb[:], hp_ps)

            # transpose to [128, q, b] via PE, fold in b_attn -> tanh bias
            for q in range(NQ):
                hpt_ps = ps_small[:, q * BC : (q + 1) * BC]
                nc.tensor.transpose(
                    hpt_ps, hp_sb[:, q * 128 : (q + 1) * 128], id_sb[:]
                )
                nc.vector.tensor_scalar_add(
                    hptb[:, q, :], hpt_ps, ba_sb[:, q : q + 1]
                )
            emit_emm(0)

            # steady state, one-block skew: e-matmuls of block i+1 sit ahead of
            # block i's tanh-dependent v-dots in the PE stream
            for i in range(NBLOCKS):
                if i + 2 < NBLOCKS:
                    load_block(i + 2)
                if i + 1 < NBLOCKS:
                    emit_emm(i + 1)
                emit_tanh(i)
                if i >= 1:
                    emit_v(i - 1)
                    emit_exp(i - 1)
            emit_v(NBLOCKS - 1)
            emit_exp(NBLOCKS - 1)

    nc.compile()
    return nc


def _get_nc():
    global _NC_CACHE
    if _NC_CACHE is None:
        _NC_CACHE = _build()
    return _NC_CACHE


def _prep_inputs(hidden, encoder_outputs, W_attn, b_attn, v):
    f = np.float32
    W_h = np.asarray(W_attn[:DH], dtype=f)
    W_e = np.asarray(W_attn[DH:], dtype=f)
    import ml_dtypes
    bf = ml_dtypes.bfloat16
    f8 = ml_dtypes.float8_e4m3
    wh_prep = np.ascontiguousarray(W_h.reshape(KD, 128, H).transpose(1, 0, 2)).astype(bf)
    we_prep = np.clip(
        np.ascontiguousarray(W_e.reshape(KH, 128, H).transpose(1, 0, 2)) * 64.0,
        -240.0, 240.0,
    ).astype(f8)
    ba_prep = np.ascontiguousarray(np.asarray(b_attn, dtype=f).reshape(NQ, 128).T)
    v_prep = np.ascontiguousarray(np.asarray(v, dtype=f).reshape(NQ, 128).T).astype(bf)
    ident = np.eye(BC, dtype=f)
    hidden = np.asarray(hidden, dtype=f)
    encoder_outputs = np.asarray(encoder_outputs, dtype=f)

    in_maps = []
    for c in range(NCORES):
        b0 = c * BC
        hc = hidden[b0 : b0 + BC]                       # [BC, DH]
        hid_prep = np.ascontiguousarray(
            hc.T.reshape(KD, 128, BC).transpose(1, 0, 2)
        ).astype(bf)
        ec = encoder_outputs[:, b0 : b0 + BC, :]        # [S, BC, H]
        # enc_prep[b, sblk, p, k, si] = ec[sblk*SBLK+si, b, k*128+p]
        enc_prep = np.clip(
            np.ascontiguousarray(
                ec.transpose(1, 0, 2)
                .reshape(BC, NBLK, SBLK, KH, 128)
                .transpose(0, 1, 4, 3, 2)
            ),
            -240.0, 240.0,
        ).astype(ml_dtypes.float8_e4m3)
        in_maps.append(
            {
                "enc_t": enc_prep,
                "hid_t": hid_prep,
                "w_h": wh_prep,
                "w_e": we_prep,
                "b_attn": ba_prep,
                "v": v_prep,
                "ident": ident,
            }
        )
    return in_maps


def _run(inputs, trace=False, **kw):
    nc = _get_nc()
    in_maps = _prep_inputs(
        inputs["hidden"],
        inputs["encoder_outputs"],
        inputs["W_attn"],
        inputs["b_attn"],
        inputs["v"],
    )
    res = run_bass_kernel_spmd(
        nc, in_maps, core_ids=list(range(NCORES)), trace=trace, **kw
    )
    out = np.concatenate([r["out"] for r in res.results], axis=0).astype(np.float32)
    return out, res


def kernel(**inputs):
    out, _ = _run(inputs, trace=False)
    return out



# revision 18
# speedup vs baseline: 1.0069x; 1.0069x over previous
"""Bahdanau-attention kernel for one TRN2 chip (8 NeuronCores, SPMD).

Math (per batch row b, sequence position s):
    att[b, s] = v . tanh(h_part[b] + enc[s, b, :] @ W_e)
    out[b, :] = softmax(att[b, :])        with h_part = hidden @ W_h + b_attn

Sharding: pure data-parallel over batch (B=32 -> 4 per core), no collectives.

v2 design ("transposed energy layout"):
- The energy matmul runs with enc as the STATIONARY operand and W_e as the
  MOVING operand, so PSUM holds energy as [128 s-positions, 512 h].  fp8
  e4m3 DoubleRow (W_e and enc pre-scaled/clipped on host), 2 matmuls per
  s-tile of 128 positions.
- h_part (+b_attn) is folded in by PRE-SEEDING each PSUM bank with a tiny
  K=5 bf16 matmul (selector rows x hp/ba rows) before the e-matmuls
  accumulate on top (start=False).  This keeps ACT bias-free.
- tanh runs on the scalar engine over [128, 1024] PSUM tiles (two s-tiles
  per op), scale=1/64 undoes the fp8 scaling.  Output bf16 to SBUF.
- The v-dot runs on the VECTOR engine as one fused tensor_tensor_reduce
  per s-tile: accum_out[p] = sum_h tanh_en[p, h] * v[h].  This removes
  ~14us of v-dot matmuls from the PE stream entirely.
- Logits accumulate into att_all[128, 4, 16] (partition = s%128).  The
  softmax is a single batched epilogue: one exp over [128, 64], per-row
  free-axis reduce, an all-ones matmul to broadcast row sums across
  partitions, one reciprocal, 4 scales, 4 PE transposes, contiguous DMA.
"""

import os
import sys

sys.path.insert(0, "/opt/trn_rl_repo")

import numpy as np

_BISECT = os.environ.get("BISECT", "")

from concourse import bacc, bass, mybir, tile
from concourse.bass_utils import run_bass_kernel_spmd
from concourse.dve_ops import TENSOR_TENSOR_REDUCE as TTR_OP

H = 512
DH = 4 * H            # 2048 (hidden feature dim)
B, S = 32, 2048
NCORES = 8
BC = B // NCORES      # 4 batch rows per core
KH = H // 128         # 4 contraction tiles over H
KD = DH // 128        # 16 contraction tiles over DH
SBLK = 1024           # sequence positions per block
NBLK = S // SBLK      # 2 blocks per batch row
NPAIR = SBLK // 256   # 4 psum pairs (2 s-tiles each) per block
NT = S // 128         # 16 s-tiles per batch row
F32 = mybir.dt.float32
BF16 = mybir.dt.bfloat16
F8 = mybir.dt.float8e4
WE_SCALE = 64.0

_NC_CACHE = None


def _build():
    nc = bacc.Bacc(
        "TRN2", target_bir_lowering=False, debug=False, num_devices=NCORES
    )
    enc_d = nc.dram_tensor(
        "enc_t", [BC, NBLK, 128, KH, SBLK], F8, kind="ExternalInput"
    )
    hid_d = nc.dram_tensor("hid_t", [128, KD, BC], BF16, kind="ExternalInput")
    wh_d = nc.dram_tensor("w_h", [128, KD, H], BF16, kind="ExternalInput")
    we_d = nc.dram_tensor("w_e", [128, KH, H], F8, kind="ExternalInput")
    ba_d = nc.dram_tensor("ba64", [BC, H], F32, kind="ExternalInput")
    st_d = nc.dram_tensor("seed_st", [BC, BC, 128], BF16, kind="ExternalInput")
    v_d = nc.dram_tensor("v_bc", [128, H], BF16, kind="ExternalInput")
    id_d = nc.dram_tensor("ident", [128, 128], F32, kind="ExternalInput")
    out_d = nc.dram_tensor("out", [BC, NT, 128], F32, kind="ExternalOutput")

    TANH = mybir.ActivationFunctionType.Tanh
    EXP = mybir.ActivationFunctionType.Exp
    MULT = mybir.AluOpType.mult
    ADD = mybir.AluOpType.add

    with tile.TileContext(nc) as tc:
        with (
            tc.tile_pool(name="const", bufs=1) as constp,
            tc.tile_pool(name="enc", bufs=6) as encp,
            tc.tile_pool(name="energy", bufs=4) as enp,
            tc.tile_pool(name="scratch", bufs=2) as scrp,
            tc.tile_pool(name="psum_e", bufs=3, space=bass.MemorySpace.PSUM) as pse,
            tc.tile_pool(name="psum_s", bufs=1, space=bass.MemorySpace.PSUM) as pss,
        ):
            wh_sb = constp.tile([128, KD, H], BF16)
            nc.scalar.dma_start(wh_sb[:, 0 : KD // 2, :], wh_d[:, 0 : KD // 2, :])
            we_sb = constp.tile([128, KH, H], F8)
            for k in range(KH):
                nc.scalar.dma_start(we_sb[:, k, :], we_d[:, k, :])
            ba_sb = constp.tile([BC, H], F32)
            nc.scalar.dma_start(ba_sb[:], ba_d[:])
            st_sb = constp.tile([BC, BC, 128], BF16)
            nc.scalar.dma_start(st_sb[:], st_d[:])
            v_sb = constp.tile([128, H], BF16)
            nc.scalar.dma_start(v_sb[:], v_d[:])
            id_sb = constp.tile([128, 128], F32)
            nc.scalar.dma_start(id_sb[:], id_d[:])

            att_all = constp.tile([128, BC, NT], F32)
            hpba = constp.tile([BC, H], BF16)
            ones_sb = constp.tile([128, 128], F32)
            nc.vector.memset(ones_sb[:], 1.0)
            ex = constp.tile([128, BC, NT], F32)
            outn = constp.tile([128, BC, NT], F32)
            sums = constp.tile([128, BC], F32)
            rs_all = constp.tile([128, BC], F32)

            ps_small = pss.tile([128, 512], F32)

            # HAM pre-warm: dummy matmuls on zeroed scratch while the first
            # DMAs are in flight, so real matmuls start at full clock
            warm = constp.tile([128, 512], BF16)
            nc.vector.memset(warm[:], 0.0)
            for _ in range(8):
                nc.tensor.matmul(
                    ps_small[:, :], warm[:, 0:128], warm[:], start=True, stop=True
                )

            blocks = [(b, sblk) for b in range(BC) for sblk in range(NBLK)]
            NBLOCKS = len(blocks)
            ets = {}

            def load_block(i):
                b, sblk = blocks[i]
                et = encp.tile([128, KH, SBLK], F8)
                nc.sync.dma_start(et[:], enc_d[b, sblk])
                ets[i] = et

            # prologue: h_part matmuls run during the first enc DMAs
            load_block(0)
            hid_sb = constp.tile([128, KD, BC], BF16)
            nc.sync.dma_start(hid_sb[:], hid_d[:])
            nc.sync.dma_start(wh_sb[:, KD // 2 :, :], wh_d[:, KD // 2 :, :])
            load_block(1)

            hp_ps = ps_small[0:BC, 0:H]
            for k in range(KD):
                nc.tensor.matmul(
                    hp_ps,
                    hid_sb[:, k, :],
                    wh_sb[:, k, :],
                    start=(k == 0),
                    stop=(k == KD - 1),
                )
            # hpba rows 0-3 = (h_part * 64) + (b_attn * 64)
            hp_tmp = constp.tile([BC, H], F32)
            nc.vector.tensor_scalar_mul(hp_tmp[:], hp_ps, WE_SCALE)
            nc.vector.tensor_add(hpba[:], hp_tmp[:], ba_sb[:])

            # pair = one [128, 1024] psum tile = 2 s-tiles of 128 positions
            pairs = [
                (i, b, sblk, q)
                for i, (b, sblk) in enumerate(blocks)
                for q in range(NPAIR)
            ]
            NP = len(pairs)
            pstiles = {}
            entiles = {}

            def emit_pe(p):
                blk, b, sblk, q = pairs[p]
                et = ets[blk]
                ps = pse.tile([128, 1024], F32, name="eps", tag="eps")
                for half in range(2):
                    s0 = 256 * q + 128 * half
                    hsl = ps[:, 512 * half : 512 * half + 512]
                    seed = _BISECT != "noseed"
                    if seed:
                        # bias seed: out[p, h] = h_part[b, h]*64 + ba[h]*64
                        nc.tensor.matmul(
                            hsl,
                            st_sb[:, b, :],
                            hpba[:],
                            start=True,
                            stop=False,
                        )
                    for j in range(KH // 2):
                        nc.tensor.matmul(
                            hsl,
                            et[:, 2 * j : 2 * j + 2, s0 : s0 + 128],
                            we_sb[:, 2 * j : 2 * j + 2, :],
                            start=(not seed and j == 0),
                            stop=(j == KH // 2 - 1),
                            perf_mode=mybir.MatmulPerfMode.DoubleRow,
                        )
                pstiles[p] = ps
                if q == NPAIR - 1:
                    ets.pop(blk)

            def emit_tanh(p):
                en = enp.tile([128, 1024], BF16)
                nc.scalar.activation(
                    en[:], pstiles.pop(p)[:], TANH, scale=1.0 / WE_SCALE
                )
                entiles[p] = en

            def emit_vdot(p):
                blk, b, sblk, q = pairs[p]
                en = entiles.pop(p)
                for half in range(2):
                    t = sblk * (SBLK // 128) + 2 * q + half
                    if _BISECT == "nottr":
                        nc.vector.reduce_sum(
                            att_all[:, b, t : t + 1],
                            en[:, 512 * half : 512 * half + 512],
                            axis=mybir.AxisListType.X,
                        )
                        continue
                    scr = scrp.tile([128, 512], BF16)
                    # custom-DVE TTR: out = in0*in1*s1, accum = s0 + sum(out)
                    # (the _custom_dve path ships the uop table in the NEFF;
                    # the plain TPB opcode hangs hardware in this runtime)
                    nc.vector._custom_dve(
                        TTR_OP,
                        out=scr[:],
                        in0=en[:, 512 * half : 512 * half + 512],
                        in1=v_sb[:],
                        s0=0.0,
                        s1=1.0,
                        accum_out=att_all[:, b, t : t + 1],
                    )

            # steady state: PE leads ACT by 1 pair, DVE by 2
            for p in range(NP):
                blk = pairs[p][0]
                if pairs[p][3] == 0 and blk + 2 < NBLOCKS:
                    load_block(blk + 2)
                emit_pe(p)
                if p >= 1:
                    emit_tanh(p - 1)
                if p >= 2:
                    emit_vdot(p - 2)
            emit_tanh(NP - 1)
            emit_vdot(NP - 2)
            emit_vdot(NP - 1)

            # softmax epilogue (batched across all 4 rows)
            nc.scalar.activation(ex[:], att_all[:], EXP)
            for b in range(BC):
                nc.vector.reduce_sum(
                    sums[:, b : b + 1], ex[:, b, :], axis=mybir.AxisListType.X
                )
            rsum_ps = ps_small[:, 0:BC]
            nc.tensor.matmul(rsum_ps, ones_sb[:], sums[:], start=True, stop=True)
            nc.vector.reciprocal(rs_all[:], rsum_ps)
            for b in range(BC):
                nc.vector.tensor_scalar_mul(
                    outn[:, b, :], ex[:, b, :], rs_all[:, b : b + 1]
                )
            for b in range(BC):
                tp_ps = ps_small[0:NT, 0:128]
                nc.tensor.transpose(tp_ps, outn[:, b, :], id_sb[:])
                ob = scrp.tile([NT, 128], F32)
                nc.vector.tensor_copy(ob[:], tp_ps)
                nc.sync.dma_start(out_d[b], ob[:])

    nc.compile()
    return nc


def _get_nc():
    global _NC_CACHE
    if _NC_CACHE is None:
        _NC_CACHE = _build()
    return _NC_CACHE


def _prep_inputs(hidden, encoder_outputs, W_attn, b_attn, v):
    f = np.float32
    W_h = np.asarray(W_attn[:DH], dtype=f)
    W_e = np.asarray(W_attn[DH:], dtype=f)
    import ml_dtypes
    bf = ml_dtypes.bfloat16
    f8 = ml_dtypes.float8_e4m3
    wh_prep = np.ascontiguousarray(W_h.reshape(KD, 128, H).transpose(1, 0, 2)).astype(bf)
    we_prep = np.clip(
        np.ascontiguousarray(W_e.reshape(KH, 128, H).transpose(1, 0, 2)) * WE_SCALE,
        -240.0, 240.0,
    ).astype(f8)
    ba_prep = np.broadcast_to(
        (np.asarray(b_attn, dtype=f) * WE_SCALE)[None, :], (BC, H)
    ).astype(f).copy()
    # seed stationary [K=BC, b, 128]: row b' = [b'==b] so the K=4 seed matmul
    # broadcasts hpba[b] across all 128 output partitions
    st = np.zeros((BC, BC, 128), dtype=f)
    for b in range(BC):
        st[b, b, :] = 1.0
    st_prep = st.astype(bf)
    v_prep = np.broadcast_to(np.asarray(v, dtype=f)[None, :], (128, H)).astype(bf).copy()
    ident = np.eye(128, dtype=f)
    hidden = np.asarray(hidden, dtype=f)
    encoder_outputs = np.asarray(encoder_outputs, dtype=f)

    in_maps = []
    for c in range(NCORES):
        b0 = c * BC
        hc = hidden[b0 : b0 + BC]                       # [BC, DH]
        hid_prep = np.ascontiguousarray(
            hc.T.reshape(KD, 128, BC).transpose(1, 0, 2)
        ).astype(bf)
        ec = encoder_outputs[:, b0 : b0 + BC, :]        # [S, BC, H]
        # enc_prep[b, sblk, p, k, si] = ec[sblk*SBLK+si, b, k*128+p]
        enc_prep = np.clip(
            np.ascontiguousarray(
                ec.transpose(1, 0, 2)
                .reshape(BC, NBLK, SBLK, KH, 128)
                .transpose(0, 1, 4, 3, 2)
            ),
            -240.0, 240.0,
        ).astype(f8)
        in_maps.append(
            {
                "enc_t": enc_prep,
                "hid_t": hid_prep,
                "w_h": wh_prep,
                "w_e": we_prep,
                "ba64": ba_prep,
                "seed_st": st_prep,
                "v_bc": v_prep,
                "ident": ident,
            }
        )
    return in_maps


def _run(inputs, trace=False, **kw):
    nc = _get_nc()
    in_maps = _prep_inputs(
        inputs["hidden"],
        inputs["encoder_outputs"],
        inputs["W_attn"],
        inputs["b_attn"],
        inputs["v"],
    )
    res = run_bass_kernel_spmd(
        nc, in_maps, core_ids=list(range(NCORES)), trace=trace, **kw
    )
    out = np.concatenate(
        [r["out"].reshape(BC, S) for r in res.results], axis=0
    ).astype(np.float32)
    return out, res


def kernel(**inputs):
    out, _ = _run(inputs, trace=False)
    return out


# revision 23
# speedup vs baseline: 1.0257x; 1.0186x over previous
"""Bahdanau-attention kernel for one TRN2 chip (8 NeuronCores, SPMD).

Math (per batch row b, sequence position s):
    att[b, s] = v . tanh(h_part[b] + enc[s, b, :] @ W_e)
    out[b, :] = softmax(att[b, :])        with h_part = hidden @ W_h + b_attn

Sharding: pure data-parallel over batch (B=32 -> 4 per core), no collectives.

v3 design ("transposed energy layout"):
- Energy matmul: enc STATIONARY, W_e MOVING -> PSUM [128 s-positions, 512 h].
  fp8 e4m3 DoubleRow, 2 matmuls per s-tile of 128 positions.
- h_part (+b_attn) is folded in per PSUM bank by a tiny K=4 bf16 seed matmul
  AFTER the e-matmuls (start=False accumulate).  The 4 seeds of a quad (4
  s-tiles) are packed into the 4 PE row-groups via tile_position so they run
  concurrently (~1 matmul span instead of 4).  Seeds-last also lets block 0
  e-matmuls issue before W_h/h_part are even loaded.
- tanh on ScalarE over [128, 1024] PSUM tiles, no bias, scale=1/64.
- v-dot on VectorE: one fused custom-DVE tensor_tensor_reduce per s-tile
  (accum_out[p] = sum_h tanh_en[p,h]*v[h]).  The _custom_dve path ships the
  uop table in the NEFF (the plain TPB opcode hangs this runtime).
- Softmax epilogue runs per batch row as soon as its 16 logit columns are
  done, overlapping the remaining rows' steady state: exp [128,16], free-axis
  reduce, all-ones matmul broadcasts the partition-sum, reciprocal, scale,
  PE transpose, contiguous [16,128] DMA out.
- DMA priority: enc block0 + W_e land first so the PE never idles at start;
  W_h halves trail on both queues (h_part isn't needed until the seeds).
"""

import os
import sys

sys.path.insert(0, "/opt/trn_rl_repo")

import numpy as np

from concourse import bacc, bass, mybir, tile
from concourse.bass_utils import run_bass_kernel_spmd
from concourse.dve_ops import TENSOR_TENSOR_REDUCE as TTR_OP

H = 512
DH = 4 * H            # 2048 (hidden feature dim)
B, S = 32, 2048
NCORES = 8
BC = B // NCORES      # 4 batch rows per core
KH = H // 128         # 4 contraction tiles over H
KD = DH // 128        # 16 contraction tiles over DH
SBLK = 1024           # sequence positions per block
NBLK = S // SBLK      # 2 blocks per batch row
NT = S // 128         # 16 s-tiles per batch row
F32 = mybir.dt.float32
BF16 = mybir.dt.bfloat16
F8 = mybir.dt.float8e4
WE_SCALE = 64.0

_NC_CACHE = None


def _build():
    nc = bacc.Bacc(
        "TRN2", target_bir_lowering=False, debug=False, num_devices=NCORES
    )
    enc_d = nc.dram_tensor(
        "enc_t", [BC, NBLK, 128, KH, SBLK], F8, kind="ExternalInput"
    )
    hid_d = nc.dram_tensor("hid_t", [128, KD, BC], BF16, kind="ExternalInput")
    wh_d = nc.dram_tensor("w_h", [128, KD, H], BF16, kind="ExternalInput")
    we_d = nc.dram_tensor("w_e", [128, KH, H], F8, kind="ExternalInput")
    ba_d = nc.dram_tensor("ba64", [BC, H], F32, kind="ExternalInput")
    st4_d = nc.dram_tensor("seed_st4", [128, BC, 128], BF16, kind="ExternalInput")
    rsel_d = nc.dram_tensor("repsel", [BC, 128], BF16, kind="ExternalInput")
    v_d = nc.dram_tensor("v_bc", [128, H], BF16, kind="ExternalInput")
    id_d = nc.dram_tensor("ident", [128, 128], F32, kind="ExternalInput")
    out_d = nc.dram_tensor("out", [BC, NT, 128], F32, kind="ExternalOutput")

    TANH = mybir.ActivationFunctionType.Tanh
    EXP = mybir.ActivationFunctionType.Exp
    DR = mybir.MatmulPerfMode.DoubleRow

    with tile.TileContext(nc) as tc:
        with (
            tc.tile_pool(name="const", bufs=1) as constp,
            tc.tile_pool(name="enc", bufs=6) as encp,
            tc.tile_pool(name="energy", bufs=4) as enp,
            tc.tile_pool(name="scratch", bufs=2) as scrp,
            tc.tile_pool(name="psum_e", bufs=4, space=bass.MemorySpace.PSUM) as pse,
        ):
            # --- DMA priority: what the PE needs first goes first ---
            # sync queue: enc block 0, hid, enc block 1, then W_h half 2
            # scalar queue: W_e, small consts, then W_h half 1
            ets = {}

            def load_block(i):
                b, sblk = blocks[i]
                et = encp.tile([128, KH, SBLK], F8)
                nc.sync.dma_start(et[:], enc_d[b, sblk])
                ets[i] = et

            blocks = [(b, sblk) for b in range(BC) for sblk in range(NBLK)]
            NBLOCKS = len(blocks)

            load_block(0)
            hid_sb = constp.tile([128, KD, BC], BF16)
            nc.sync.dma_start(hid_sb[:], hid_d[:])
            load_block(1)
            wh_sb = constp.tile([128, KD, H], BF16)
            nc.sync.dma_start(wh_sb[:, KD // 2 :, :], wh_d[:, KD // 2 :, :])

            we_sb = constp.tile([128, KH, H], F8)
            for k in range(KH):
                nc.scalar.dma_start(we_sb[:, k, :], we_d[:, k, :])
            ba_sb = constp.tile([BC, H], F32)
            nc.scalar.dma_start(ba_sb[:], ba_d[:])
            st4_sb = constp.tile([128, BC, 128], BF16)
            nc.scalar.dma_start(st4_sb[:], st4_d[:])
            rsel_sb = constp.tile([BC, 128], BF16)
            nc.scalar.dma_start(rsel_sb[:], rsel_d[:])
            v_sb = constp.tile([128, H], BF16)
            nc.scalar.dma_start(v_sb[:], v_d[:])
            id_sb = constp.tile([128, 128], F32)
            nc.scalar.dma_start(id_sb[:], id_d[:])
            nc.scalar.dma_start(wh_sb[:, 0 : KD // 2, :], wh_d[:, 0 : KD // 2, :])

            att_all = constp.tile([128, BC, NT], F32)
            hpstage = constp.tile([BC, H], BF16)
            hp_tmp = constp.tile([BC, H], F32)
            hpba4 = constp.tile([128, H], BF16)
            ones_sb = constp.tile([128, 128], F32)
            nc.vector.memset(ones_sb[:], 1.0)
            ex = constp.tile([128, BC, NT], F32)
            outn = constp.tile([128, BC, NT], F32)
            sums = constp.tile([128, BC], F32)
            rs_all = constp.tile([128, BC], F32)

            # HAM pre-warm on a psum tile while the first DMAs land
            warm = constp.tile([128, 512], BF16)
            nc.vector.memset(warm[:], 0.0)
            warm_ps = pse.tile([128, 1024], F32, tag="eps")
            for _ in range(8):
                nc.tensor.matmul(
                    warm_ps[:, 0:512], warm[:, 0:128], warm[:], start=True, stop=True
                )

            # quad = 4 s-tiles = 2 psum pair-tiles; pairs trail for tanh/vdot
            NQUAD = NBLOCKS * 2          # 16 quads, each 256 s-positions
            pstiles = {}
            entiles = {}

            def emit_emm_quad(Q):
                blk = Q // 2
                b, sblk = blocks[blk]
                et = ets[blk]
                ps0 = pse.tile([128, 1024], F32, name="eps0", tag="eps")
                ps1 = pse.tile([128, 1024], F32, name="eps1", tag="eps")
                for rg in range(4):
                    ps = ps0 if rg < 2 else ps1
                    s0 = 512 * (Q % 2) + 128 * rg
                    hsl = ps[:, 512 * (rg % 2) : 512 * (rg % 2) + 512]
                    for j in range(KH // 2):
                        nc.tensor.matmul(
                            hsl,
                            et[:, 2 * j : 2 * j + 2, s0 : s0 + 128],
                            we_sb[:, 2 * j : 2 * j + 2, :],
                            start=(j == 0),
                            stop=False,
                            perf_mode=DR,
                        )
                pstiles[2 * Q] = ps0
                pstiles[2 * Q + 1] = ps1
                if Q % 2 == 1:
                    ets.pop(blk)

            def emit_seed_quad(Q):
                blk = Q // 2
                b, sblk = blocks[blk]
                # 4 concurrent K=4 seeds, one per PE row-group (tile_position)
                for rg in range(4):
                    ps = pstiles[2 * Q] if rg < 2 else pstiles[2 * Q + 1]
                    hsl = ps[:, 512 * (rg % 2) : 512 * (rg % 2) + 512]
                    nc.tensor.matmul(
                        hsl,
                        st4_sb[32 * rg : 32 * rg + 4, b, :],
                        hpba4[32 * rg : 32 * rg + 4, :],
                        start=False,
                        stop=True,
                        tile_position=(32 * rg, 0),
                    )

            def emit_tanh(p):
                en = enp.tile([128, 1024], BF16)
                nc.scalar.activation(
                    en[:], pstiles.pop(p)[:], TANH, scale=1.0 / WE_SCALE
                )
                entiles[p] = en

            def emit_vdot(p):
                blk = p // 4
                b, sblk = blocks[blk]
                en = entiles.pop(p)
                for half in range(2):
                    t = sblk * (SBLK // 128) + 2 * (p % 4) + half
                    scr = scrp.tile([128, 512], BF16)
                    # custom-DVE TTR: out = in0*in1*s1, accum = s0 + sum(out)
                    nc.vector._custom_dve(
                        TTR_OP,
                        out=scr[:],
                        in0=en[:, 512 * half : 512 * half + 512],
                        in1=v_sb[:],
                        s0=0.0,
                        s1=1.0,
                        accum_out=att_all[:, b, t : t + 1],
                    )
                if p == 8 * b + 7:
                    emit_softmax(b)

            def emit_softmax(b):
                # row b's 16 logit columns are complete: softmax + store
                eps = pse.tile([128, 1024], F32, name="soft", tag="eps")
                nc.scalar.activation(ex[:, b, :], att_all[:, b, :], EXP)
                nc.vector.reduce_sum(
                    sums[:, b : b + 1], ex[:, b, :], axis=mybir.AxisListType.X
                )
                rs_ps = eps[:, 0:1]
                nc.tensor.matmul(
                    rs_ps, ones_sb[:], sums[:, b : b + 1], start=True, stop=True
                )
                nc.vector.reciprocal(rs_all[:, b : b + 1], rs_ps)
                nc.vector.tensor_scalar_mul(
                    outn[:, b, :], ex[:, b, :], rs_all[:, b : b + 1]
                )
                tp_ps = eps[0:NT, 512:640]
                nc.tensor.transpose(tp_ps, outn[:, b, :], id_sb[:])
                ob = scrp.tile([NT, 128], F32)
                nc.vector.tensor_copy(ob[:], tp_ps)
                nc.sync.dma_start(out_d[b], ob[:])

            # prologue PE work after warm: block0 e-matmuls, h_part chain
            # (emitted before quad1 so warm_ps's buffer is reuse-safe), then
            # block1 e-matmuls while the DVE prepares hpba4, then the seeds
            emit_emm_quad(0)

            hp_ps = warm_ps[0:BC, 0:H]
            for k in range(KD):
                nc.tensor.matmul(
                    hp_ps,
                    hid_sb[:, k, :],
                    wh_sb[:, k, :],
                    start=(k == 0),
                    stop=(k == KD - 1),
                )
            nc.vector.tensor_scalar_mul(hp_tmp[:], hp_ps, WE_SCALE)
            nc.vector.tensor_add(hpstage[:], hp_tmp[:], ba_sb[:])
            rep_ps = warm_ps[:, 0:512]
            nc.tensor.matmul(rep_ps, rsel_sb[:], hpstage[:], start=True, stop=True)
            nc.vector.tensor_copy(hpba4[:], rep_ps)
            load_block(2)

            emit_emm_quad(1)
            emit_seed_quad(0)
            emit_seed_quad(1)

            # steady state: PE quads lead; tanh trails ~2 pairs, vdot ~4
            emit_tanh(0)
            for Q in range(2, NQUAD):
                blk = Q // 2
                if Q % 2 == 0 and blk + 2 < NBLOCKS:
                    load_block(blk + 2)
                emit_emm_quad(Q)
                emit_seed_quad(Q)
                emit_tanh(2 * Q - 3)
                emit_tanh(2 * Q - 2)
                if 2 * Q - 5 >= 0:
                    emit_vdot(2 * Q - 5)
                if 2 * Q - 4 >= 0:
                    emit_vdot(2 * Q - 4)
            for p in range(2 * NQUAD - 3, 2 * NQUAD):
                emit_tanh(p)
            for p in range(2 * NQUAD - 5, 2 * NQUAD):
                emit_vdot(p)

    nc.compile()
    return nc


def _get_nc():
    global _NC_CACHE
    if _NC_CACHE is None:
        _NC_CACHE = _build()
    return _NC_CACHE


def _prep_inputs(hidden, encoder_outputs, W_attn, b_attn, v):
    f = np.float32
    W_h = np.asarray(W_attn[:DH], dtype=f)
    W_e = np.asarray(W_attn[DH:], dtype=f)
    import ml_dtypes
    bf = ml_dtypes.bfloat16
    f8 = ml_dtypes.float8_e4m3
    wh_prep = np.ascontiguousarray(W_h.reshape(KD, 128, H).transpose(1, 0, 2)).astype(bf)
    we_prep = np.clip(
        np.ascontiguousarray(W_e.reshape(KH, 128, H).transpose(1, 0, 2)) * WE_SCALE,
        -240.0, 240.0,
    ).astype(f8)
    ba_prep = np.broadcast_to(
        (np.asarray(b_attn, dtype=f) * WE_SCALE)[None, :], (BC, H)
    ).astype(f).copy()
    # seed stationary [p, b, 128]: 1 iff p%32 == b, so the [4,128] slice at
    # base partition 32*rg selects hpba4 row 32*rg+b for row-group rg
    pidx = np.arange(128)
    st4 = np.zeros((128, BC, 128), dtype=f)
    for b in range(BC):
        st4[pidx % 32 == b, b, :] = 1.0
    st4_prep = st4.astype(bf)
    # repsel[b, p] = 1 iff p%32 == b: replicates hpstage rows 0-3 to
    # partitions {32rg+b} of hpba4
    rsel = np.zeros((BC, 128), dtype=f)
    for b in range(BC):
        rsel[b, pidx % 32 == b] = 1.0
    rsel_prep = rsel.astype(bf)
    v_prep = np.broadcast_to(np.asarray(v, dtype=f)[None, :], (128, H)).astype(bf).copy()
    ident = np.eye(128, dtype=f)
    hidden = np.asarray(hidden, dtype=f)
    encoder_outputs = np.asarray(encoder_outputs, dtype=f)

    in_maps = []
    for c in range(NCORES):
        b0 = c * BC
        hc = hidden[b0 : b0 + BC]                       # [BC, DH]
        hid_prep = np.ascontiguousarray(
            hc.T.reshape(KD, 128, BC).transpose(1, 0, 2)
        ).astype(bf)
        ec = encoder_outputs[:, b0 : b0 + BC, :]        # [S, BC, H]
        # enc_prep[b, sblk, p, k, si] = ec[sblk*SBLK+si, b, k*128+p]
        enc_prep = np.clip(
            np.ascontiguousarray(
                ec.transpose(1, 0, 2)
                .reshape(BC, NBLK, SBLK, KH, 128)
                .transpose(0, 1, 4, 3, 2)
            ),
            -240.0, 240.0,
        ).astype(f8)
        in_maps.append(
            {
                "enc_t": enc_prep,
                "hid_t": hid_prep,
                "w_h": wh_prep,
                "w_e": we_prep,
                "ba64": ba_prep,
                "seed_st4": st4_prep,
                "repsel": rsel_prep,
                "v_bc": v_prep,
                "ident": ident,
            }
        )
    return in_maps


def _run(inputs, trace=False, **kw):
    nc = _get_nc()
    in_maps = _prep_inputs(
        inputs["hidden"],
        inputs["encoder_outputs"],
        inputs["W_attn"],
        inputs["b_attn"],
        inputs["v"],
    )
    res = run_bass_kernel_spmd(
        nc, in_maps, core_ids=list(range(NCORES)), trace=trace, **kw
    )
    out = np.concatenate(
        [r["out"].reshape(BC, S) for r in res.results], axis=0
    ).astype(np.float32)
    return out, res


def kernel(**inputs):
    out, _ = _run(inputs, trace=False)
    return out
